# revision 19
# baseline (speedup 1.0000x reference)
"""ConvTransE forward on 8 Trainium2 NeuronCores (Bass/Tile).

Math shortcut: the reference computes scores = x @ ent.T  ([B, 100000]) and
returns scores[i, t[i]]; we only compute out[b] = x[b] . ent[t[b]].
The conv's retained slice [:, :512] depends only on ent[h] and rel[r][:, 0].

Sharding: 2-way tensor-parallel over conv channels x 4-way data-parallel
over batch.  Core m owns channels [16*(m//4), 16*(m//4)+16) and batch tiles
[4*(m%4), 4*(m%4)+4) (128 rows each).  Every core:
  - gathers ent[h] and ent[t] rows (bf16) for its 4 batch tiles,
  - PE-transposes overlapping 128-wide windows of the gathered rows,
  - runs the conv as banded bf16 matmuls on the PE (one shared band matrix
    per channel; the rel[r][:,0] contraction row is host-prepared),
  - projects its half of the contraction:  z = relu(conv) @ proj_w_g^T,
  - emits partial[b] = z[b] . ent[t[b]] via a fused multiply+row-sum.
proj_b rides along as a K=1 "ones" matmul fed zeros on the G=1 cores.
Host sums the 2 channel-half partials per batch tile.

Scheduling: a short burst of dummy matmuls trips the PE HAM clock gate
while the first gathers are in flight; projection weights ride HWDGE with
wait_until floors so they stay behind the gathers; relu+bias is split
across the Scalar, Vector, and GpSimd engines to keep pace with the PE.
"""

import numpy as np

NE, NRR, D, C, B = 100000, 500, 512, 32, 2048
NCORES = 8
NBG = 4                    # batch groups (cores m%4)
NCG = 2                    # channel groups (cores m//4)
CPC = C // NCG             # 16 channels per core
NT = B // 128 // NBG       # 4 batch tiles of 128 per core
NB = NT * 128              # 512 batch columns per core
JB = 126                   # conv j-block (126 outputs need a 128-wide window)
NSEG = CPC * 4             # 64 (c, s) main contraction blocks per core
NCHUNK = 16                # pwt DMA chunks (4 blocks each)
NWU = 28                   # PE warm-up matmuls (HAM clock-gate trip)

_CACHE = {}


def _build_nc(mode="full"):
    from contextlib import ExitStack

    import concourse.bass as bass
    import concourse.tile as tile
    from concourse import bacc, mybir
    from concourse.masks import make_identity

    f32 = mybir.dt.float32
    bf16 = mybir.dt.bfloat16
    i32 = mybir.dt.int32
    Alu = mybir.AluOpType

    nc = bacc.Bacc("TRN2", target_bir_lowering=False, debug=False,
                   num_devices=NCORES)

    ent = nc.dram_tensor("ent", [NE, D], bf16, kind="ExternalInput")
    idx = nc.dram_tensor("idx", [128, 2 * NT], i32, kind="ExternalInput")
    relrow = nc.dram_tensor("relrow", [1, NB], bf16, kind="ExternalInput")
    band = nc.dram_tensor("band", [128, 2 * CPC * JB + 128], bf16,
                          kind="ExternalInput")
    pwts = [nc.dram_tensor(f"pwt{k}", [JB, (NSEG // NCHUNK) * D], bf16,
                           kind="ExternalInput") for k in range(NCHUNK)]
    pstub = nc.dram_tensor("pstub", [128, 2 * D], bf16, kind="ExternalInput")
    cpack = nc.dram_tensor("cpack", [128, CPC + 1], f32, kind="ExternalInput")
    out = nc.dram_tensor("out", [128, NT], f32, kind="ExternalOutput")

    nrep = 1
    if mode.startswith("x"):
        nrep = int(mode[1:])
        mode = "full"

    with tile.TileContext(nc) as tc, ExitStack() as ctx:
        const = ctx.enter_context(tc.tile_pool(name="const", bufs=1))
        gpad_p = ctx.enter_context(tc.tile_pool(name="gpad", bufs=4))
        vt_p = ctx.enter_context(tc.tile_pool(name="vt", bufs=4))
        ym_p = ctx.enter_context(tc.tile_pool(name="ym", bufs=1))
        sc_p = ctx.enter_context(tc.tile_pool(name="scr", bufs=2))
        tp_p = ctx.enter_context(tc.tile_pool(name="tp", bufs=1, space="PSUM"))
        ts_p = ctx.enter_context(tc.tile_pool(name="ts", bufs=1, space="PSUM"))
        yp_p = ctx.enter_context(tc.tile_pool(name="yp", bufs=2, space="PSUM"))
        z_p = ctx.enter_context(tc.tile_pool(name="zp", bufs=1, space="PSUM"))

        ident = const.tile([128, 128], bf16)
        make_identity(nc, ident[:])
        ones_sb = const.tile([1, 128], bf16)
        nc.vector.memset(ones_sb[:], 1.0)
        out_sb = const.tile([128, NT], f32)

        idx_sb = const.tile([128, 2 * NT], i32)
        nc.sync.dma_start(idx_sb[:], idx[:])
        band_sb = const.tile([128, 2 * CPC * JB + 128], bf16)
        cp_sb = const.tile([128, CPC + 1], f32)
        pwt_sb = [const.tile([JB, (NSEG // NCHUNK) * D], bf16, name=f"pwt{k}")
                  for k in range(NCHUNK)]
        ps_sb = const.tile([128, 2 * D], bf16)

        # Dummy matmuls: keep the PE active so the HAM clock gate lifts
        # (1.2 -> 2.4 GHz) while the first gathers are still in flight.
        wu = tp_p.tile([128, 128], f32, tag="tp", name="wu")
        for _ in range(NWU):
            nc.tensor.matmul(wu[:], ident[:], ident[:], start=True, stop=True)

        for _rep in range(nrep):
            gw = sc_p.tile([128, 4 * NB], bf16, tag="gw", name="gw")
            gws = sc_p.tile([10, NB], bf16, tag="gws", name="gws")
            ts_ps = ts_p.tile([9, NB], bf16)
            gpads, vts = [], []
            for q in range(NT):
                gpad = gpad_p.tile([128, D], bf16)
                nc.gpsimd.indirect_dma_start(
                    out=gpad[:], out_offset=None, in_=ent[:],
                    in_offset=bass.IndirectOffsetOnAxis(
                        ap=idx_sb[:, q:q + 1], axis=0))
                gpads.append(gpad)
            # biases + the rel contraction row (HWDGE; tiny)
            nc.sync.dma_start(cp_sb[:], cpack[:])
            nc.sync.dma_start(gws[9:10, :], relrow[:])
            # conv bands, gated behind the first gathers so their bulk
            # shares the SDMA engines with the remaining gathers and still
            # lands before the first conv matmul needs them
            nc.vector.tensor_copy(band_sb[0:1, 0:1], gpads[0][0:1, 0:1])
            nc.sync.dma_start(band_sb[:, 0:CPC * JB], band[:, 0:CPC * JB])
            nc.vector.tensor_copy(band_sb[0:1, CPC * JB:CPC * JB + 1],
                                  gpads[1][0:1, 0:1])
            nc.sync.dma_start(band_sb[:, CPC * JB:],
                              band[:, CPC * JB:])

            for q in range(NT):
                tp = tp_p.tile([128, 512], bf16, tag="tp")
                for s in range(4):
                    w0 = 0 if s == 0 else s * JB - 1
                    nc.tensor.transpose(tp[:, s * 128:(s + 1) * 128],
                                        gpads[q][:, w0:w0 + 128],
                                        ident[:])
                for s in range(4):
                    nc.vector.tensor_copy(
                        gw[:, s * NB + q * 128:s * NB + (q + 1) * 128],
                        tp[:, s * 128:(s + 1) * 128])
            for q in range(NT):
                nc.tensor.transpose(ts_ps[:, q * 128:(q + 1) * 128],
                                    gpads[q][:, 4 * JB - 1:D], ident[:])
            nc.vector.tensor_copy(gws[0:9, :], ts_ps[:])

            # Weight / ent[t] transfers gated on the transposed gathers: the
            # corner-copy WAW dep keeps their bulk off the DMA queue until
            # the gather-side traffic is done (robust in both the scheduler
            # and on hardware, unlike time-based floors).
            for k in range(NCHUNK):
                nc.vector.tensor_copy(pwt_sb[k][0:1, 0:1], gw[0:1, 0:1])
                nc.sync.dma_start(pwt_sb[k][:], pwts[k][:])
            nc.vector.tensor_copy(ps_sb[0:1, 0:1], gw[0:1, 0:1])
            nc.sync.dma_start(ps_sb[:], pstub[:])
            # ent[t] rows: only read by the final dot
            for q in range(NT):
                vt = vt_p.tile([128, D], bf16)
                nc.vector.tensor_copy(vt[0:1, 0:1], gw[0:1, 0:1])
                nc.gpsimd.indirect_dma_start(
                    out=vt[:], out_offset=None, in_=ent[:],
                    in_offset=bass.IndirectOffsetOnAxis(
                        ap=idx_sb[:, NT + q:NT + q + 1], axis=0))
                vts.append(vt)

            # conv and projection interleaved per (c, s) block: the PE does
            # 1 conv + 4 proj matmuls per block while Scalar/Vector apply
            # relu+bias to the next block -- keeps the PE the bottleneck.
            # The stub contraction runs first so each tile's z group opens
            # with the stub + bias matmuls and can close (and dot) as soon
            # as its last main block lands.
            ym = ym_p.tile([JB, NSEG * NB], bf16, tag="ym", name="ym")
            ystub = sc_p.tile([128, NB], bf16, tag="ystub", name="ystub")
            zs = [z_p.tile([128, D], f32, name=f"z{q}") for q in range(NT)]

            NTAIL = NSEG - 4   # last 4 blocks run tile-major, dots interleave

            def proj_block(cs, qs=range(NT)):
                ck, off = divmod(cs, NSEG // NCHUNK)
                for q in qs:
                    nc.tensor.matmul(
                        zs[q][:],
                        ym[:, cs * NB + q * 128:cs * NB + (q + 1) * 128],
                        pwt_sb[ck][:, off * D:(off + 1) * D],
                        start=(cs == 0), stop=False)

            for c in range(CPC):
                for s in range(4):
                    cs = c * 4 + s
                    yp = yp_p.tile([JB, NB], f32)
                    boff = c * JB if s == 0 else CPC * JB + c * JB
                    nc.tensor.matmul(yp[:], band_sb[:, boff:boff + JB],
                                     gw[:, s * NB:(s + 1) * NB],
                                     start=True, stop=True)
                    if cs % 2 == 0:
                        nc.scalar.activation(
                            ym[:, cs * NB:(cs + 1) * NB], yp[:],
                            mybir.ActivationFunctionType.Relu,
                            bias=cp_sb[0:JB, c:c + 1])
                    else:
                        nc.vector.tensor_scalar(ym[:, cs * NB:(cs + 1) * NB],
                                                yp[:], cp_sb[0:JB, c:c + 1],
                                                0.0, Alu.add, Alu.max)
                    # software pipeline: project the previous block while
                    # this block's relu drains on Scalar/Vector
                    if 0 < cs <= NTAIL:
                        proj_block(cs - 1)
            yps = yp_p.tile([128, NB], f32, tag="yp")
            nc.tensor.matmul(yps[:], band_sb[0:10, 2 * CPC * JB:2 * CPC * JB + 128],
                             gws[:], start=True, stop=True)
            nc.scalar.activation(ystub[:], yps[:],
                                 mybir.ActivationFunctionType.Relu,
                                 bias=cp_sb[:, CPC:CPC + 1])
            for q in range(NT):
                for cs in range(NTAIL, NSEG):
                    proj_block(cs, qs=[q])
                nc.tensor.matmul(zs[q][:], ystub[:, q * 128:(q + 1) * 128],
                                 ps_sb[:, 0:D], start=False, stop=False)
                nc.tensor.matmul(zs[q][:], ones_sb[:], ps_sb[0:1, D:2 * D],
                                 start=False, stop=True)
                scr = sc_p.tile([128, D], f32, tag="scr", name="scr")
                nc.vector.scalar_tensor_tensor(
                    out=scr[:], in0=zs[q][:], scalar=1.0, in1=vts[q][:],
                    op0=Alu.mult, op1=Alu.mult,
                    accum_out=out_sb[:, q:q + 1])

        nc.sync.dma_start(out[:], out_sb[:])
    nc.finalize()
    return nc


def _host_prep(inputs):
    """Per-core input dicts from the full problem inputs."""
    import ml_dtypes
    bf16 = ml_dtypes.bfloat16

    ent = np.ascontiguousarray(
        np.asarray(inputs["ent"], dtype=np.float32).astype(bf16))
    rel0 = np.asarray(inputs["rel"], dtype=np.float32)[:, 0]
    w = np.asarray(inputs["conv_w"], dtype=np.float32)       # [32, 1, 3]
    cb = np.asarray(inputs["conv_b"], dtype=np.float32)      # [32]
    pw = np.asarray(inputs["proj_w"], dtype=np.float32)      # [512, 16384]
    pb = np.asarray(inputs["proj_b"], dtype=np.float32)      # [512]
    h = np.asarray(inputs["h"]).astype(np.int32)
    r = np.asarray(inputs["r"]).astype(np.int32)
    t = np.asarray(inputs["t"]).astype(np.int32)

    NQ = B // 128
    hIf = np.ascontiguousarray(h.reshape(NQ, 128).T)
    tIf = np.ascontiguousarray(t.reshape(NQ, 128).T)
    relf = np.ascontiguousarray(rel0[r].reshape(NQ, 128).T)  # [128, NQ]

    jl = np.arange(JB)
    jl8 = np.arange(8)
    in_maps = []
    for m in range(NCORES):
        g, bq = m // NBG, m % NBG
        bandm = np.zeros((128, 2 * CPC * JB + 128), np.float32)
        pwt = np.zeros((JB, NSEG, D), np.float32)
        pstubm = np.zeros((128, 2 * D), np.float32)
        cpackm = np.zeros((128, CPC + 1), np.float32)
        for c in range(CPC):
            cg = g * CPC + c
            cpackm[:, c] = cb[cg]
            cpackm[c * 8:(c + 1) * 8, CPC] = cb[cg]
            for k in range(3):
                bandm[jl8 + k, 2 * CPC * JB + c * 8 + jl8] = w[cg, 0, k]
                # s0 variant: window starts at x[0] (w0 term for j=0 is 0)
                jv = jl if k > 0 else jl[1:]
                bandm[jv + k - 1, c * JB + jv] = w[cg, 0, k]
                # regular: window starts at x[base-1]
                bandm[jl + k, CPC * JB + c * JB + jl] = w[cg, 0, k]
            for s in range(4):
                cs = c * 4 + s
                pwt[:, cs, :] = pw[:, cg * D + JB * s: cg * D + JB * (s + 1)].T
            pstubm[c * 8:(c + 1) * 8, 0:D] = \
                pw[:, cg * D + 504: cg * D + 512].T
        if g == 0:
            pstubm[0, D:2 * D] = pb
        idxm = np.concatenate([hIf[:, bq * NT:(bq + 1) * NT],
                               tIf[:, bq * NT:(bq + 1) * NT]], axis=1)
        pwt = pwt.reshape(JB, NSEG * D).astype(bf16)
        im = {
            "ent": ent,
            "idx": np.ascontiguousarray(idxm),
            "relrow": np.ascontiguousarray(
                relf[:, bq * NT:(bq + 1) * NT].T.reshape(1, NB)).astype(bf16),
            "band": bandm.astype(bf16),
            "pstub": pstubm.astype(bf16),
            "cpack": cpackm,
        }
        for k in range(NCHUNK):
            nb = NSEG // NCHUNK
            im[f"pwt{k}"] = np.ascontiguousarray(
                pwt[:, k * nb * D:(k + 1) * nb * D])
        in_maps.append(im)
    return in_maps


def _run(inputs, trace=False, tmpdir=None, mode="full"):
    from concourse.bass_utils import run_bass_kernel_spmd

    if mode not in _CACHE:
        _CACHE[mode] = _build_nc(mode)
    nc = _CACHE[mode]
    in_maps = _host_prep(inputs)
    res = run_bass_kernel_spmd(nc, in_maps, core_ids=list(range(NCORES)),
                               trace=trace, tmpdir=tmpdir)
    NQ = B // 128
    total = np.zeros((128, NQ), np.float64)
    for m, mres in enumerate(res.results):
        bq = m % NBG
        total[:, bq * NT:(bq + 1) * NT] += mres["out"].astype(np.float64)
    return total.T.reshape(B).astype(np.float32), res


def kernel(**inputs):
    out, _ = _run(inputs, trace=False)
    return out
